# revision 4
# baseline (speedup 1.0000x reference)
"""Multi-head attention (B=4, S=2048, D=1024, H=16) on 8 Trainium2 cores.

Sharding: core c computes batch b = c // 2, head group hg = c % 2 (8 heads).
Each core runs the full pipeline for its (b, hg): QKV projections restricted
to its head group's rows of Wq/Wk/Wv, per-head attention, and a partial
output projection against its head group's columns of Wo. The host sums the
two partial outputs per batch (the out-projection is linear in the head dim).

On-device layouts are transposed (feature on partitions):
  qT/kT  [512, 2048]  (head-dim on partitions; head pair p at partitions
                       of tile p, head 2p rows 0:64, head 2p+1 rows 64:128)
  scores S^T [k, q] per head -> exp -> E^T used directly as the stationary
  operand of the context matmul; V is augmented with a ones column so the
  softmax denominator falls out of the same accumulation.
"""

import numpy as np

B, S, D, H = 4, 2048, 1024, 16
HD = D // H          # 64
NHL = 8              # heads per core
DHG = NHL * HD       # 512 head-group width
HDA = HD + 1         # augmented head dim (ones column)
P = 128
N_CORES = 8

_CACHE = {}


def _build_nc(debug=False):
    import concourse.bacc as bacc
    import concourse.mybir as mybir
    from concourse.tile import TileContext

    f16 = mybir.dt.float16
    f32 = mybir.dt.float32
    EXP = mybir.ActivationFunctionType.Exp

    nc = bacc.Bacc("TRN2", target_bir_lowering=False, debug=False,
                   num_devices=N_CORES)

    xqT = nc.dram_tensor("xqT", [D, S], f16, kind="ExternalInput")
    xkT = nc.dram_tensor("xkT", [D, S], f16, kind="ExternalInput")
    xvT = nc.dram_tensor("xvT", [D, S], f16, kind="ExternalInput")
    wqT = nc.dram_tensor("wqT", [D, DHG], f16, kind="ExternalInput")
    wkT = nc.dram_tensor("wkT", [D, DHG], f16, kind="ExternalInput")
    wvT = nc.dram_tensor("wvT", [D, DHG], f16, kind="ExternalInput")
    woT = nc.dram_tensor("woT", [DHG, D], f16, kind="ExternalInput")
    out = nc.dram_tensor("out", [S, D], f32, kind="ExternalOutput")
    ctx_dram = nc.dram_tensor("ctx_dram", [S, DHG], f16)
    if debug:
        dbg_qT = nc.dram_tensor("dbg_qT", [DHG, S], f16, kind="ExternalOutput")
        dbg_kT = nc.dram_tensor("dbg_kT", [DHG, S], f16, kind="ExternalOutput")
        dbg_vaug = nc.dram_tensor("dbg_vaug", [S, NHL * HDA], f16, kind="ExternalOutput")
        dbg_ctx = nc.dram_tensor("dbg_ctx", [S, DHG], f16, kind="ExternalOutput")

    DT = D // P          # 8 input-dim tiles
    PT = DHG // P        # 4 head-pair tiles
    QC = S // 512        # 4 query chunks
    KB = S // P          # 16 key chunks

    with TileContext(nc) as tc:
        with (
            tc.tile_pool(name="weights", bufs=1) as wpool,
            tc.tile_pool(name="persist", bufs=1) as persist,
            tc.tile_pool(name="xstream", bufs=12) as xpool,
            tc.tile_pool(name="evict", bufs=4) as epool,
            tc.tile_pool(name="proj_psum", bufs=2, space="PSUM") as proj_psum,
            tc.tile_pool(name="sc_psum", bufs=2, space="PSUM") as sc_psum,
            tc.tile_pool(name="ctx_psum", bufs=2, space="PSUM") as ctx_psum,
        ):
            wq = wpool.tile([P, DT, DHG], f16)
            wk = wpool.tile([P, DT, DHG], f16)
            wv = wpool.tile([P, DT, DHG], f16)
            wo = wpool.tile([P, PT, D], f16)
            for dt in range(DT):
                nc.sync.dma_start(wq[:, dt], wqT[dt * P:(dt + 1) * P, :])
                nc.sync.dma_start(wk[:, dt], wkT[dt * P:(dt + 1) * P, :])
                nc.sync.dma_start(wv[:, dt], wvT[dt * P:(dt + 1) * P, :])
            for dt in range(PT):
                nc.sync.dma_start(wo[:, dt], woT[dt * P:(dt + 1) * P, :])

            qT = persist.tile([P, PT, S], f16)
            kT = persist.tile([P, PT, S], f16)
            vaug = persist.tile([P, KB, NHL * HDA], f16)
            ctx = persist.tile([P, S // P, DHG], f16)

            # K and Q projections, head-pair-major so attention can start
            # on pair 0 as early as possible.  kT/qT tile pt holds heads
            # (2pt, 2pt+1):  kT[:, pt, s] = (x @ W.T).T rows of the pair.
            for (w, xdram, dst) in ((wk, xkT, kT), (wq, xqT, qT)):
                for pt in range(PT):
                    for qc in range(QC):
                        xs = []
                        for dt in range(DT):
                            xt = xpool.tile([P, 512], f16, tag="x")
                            nc.sync.dma_start(
                                xt[:], xdram[dt * P:(dt + 1) * P,
                                             qc * 512:(qc + 1) * 512])
                            xs.append(xt)
                        ps = proj_psum.tile([P, 512], f32, tag="proj")
                        for dt in range(DT):
                            nc.tensor.matmul(
                                ps[:], w[:, dt, pt * P:(pt + 1) * P], xs[dt][:],
                                start=(dt == 0), stop=(dt == DT - 1))
                        nc.vector.tensor_copy(
                            dst[:, pt, qc * 512:(qc + 1) * 512], ps[:])

            # V projection into natural [key, head-dim] layout, augmented
            # with a ones column per head for the softmax denominator.
            for kc in range(QC):
                xs = []
                for dt in range(DT):
                    xt = xpool.tile([P, 512], f16, tag="x")
                    nc.sync.dma_start(
                        xt[:], xvT[dt * P:(dt + 1) * P, kc * 512:(kc + 1) * 512])
                    xs.append(xt)
                for ks in range(4):
                    kb = kc * 4 + ks
                    ps = proj_psum.tile([P, 512], f32, tag="proj")
                    for dt in range(DT):
                        nc.tensor.matmul(
                            ps[:], xs[dt][:, ks * P:(ks + 1) * P], wv[:, dt],
                            start=(dt == 0), stop=(dt == DT - 1))
                    va = vaug[:, kb].rearrange("p (h x) -> p h x", h=NHL)
                    nc.any.memset(va[:, :, HD:HDA], 1.0)
                    nc.vector.tensor_copy(
                        va[:, :, 0:HD],
                        ps[:].rearrange("p (h x) -> p h x", h=NHL))

            # Attention: per head pair and query chunk, stream key chunks.
            for pr in range(PT):
                for qc in range(QC):
                    cps = [ctx_psum.tile([P, 4 * HDA], f32, tag="ctx",
                                         name=f"ctx_{pr}_{qc}_{h}")
                           for h in range(2)]
                    for kb in range(KB):
                        sc = sc_psum.tile([P, 1024], f32, tag="sc")
                        et = epool.tile([P, 1024], f16, tag="e")
                        for h in range(2):
                            nc.tensor.matmul(
                                sc[:, h * 512:(h + 1) * 512],
                                kT[h * 64:(h + 1) * 64, pr, kb * P:(kb + 1) * P],
                                qT[h * 64:(h + 1) * 64, pr, qc * 512:(qc + 1) * 512],
                                start=True, stop=True)
                        nc.scalar.activation(et[:], sc[:], EXP, scale=1.0 / 8.0)
                        for h in range(2):
                            hg = 2 * pr + h
                            for qs in range(4):
                                # exactly one start/stop per PSUM bank: start
                                # clears the whole bank's has_written bits
                                nc.tensor.matmul(
                                    cps[h][:, qs * HDA:(qs + 1) * HDA],
                                    et[:, h * 512 + qs * P: h * 512 + (qs + 1) * P],
                                    vaug[:, kb, hg * HDA:(hg + 1) * HDA],
                                    start=(kb == 0 and qs == 0),
                                    stop=(kb == KB - 1 and qs == 3),
                                    skip_group_check=True)
                    for h in range(2):
                        hg = 2 * pr + h
                        rec = epool.tile([P, 4], f32, tag="rec")
                        den = cps[h][:].rearrange(
                            "p (qs x) -> p qs x", qs=4)[:, :, HD:HDA]
                        nc.vector.reciprocal(rec[:], den)
                        for qs in range(4):
                            nc.vector.tensor_scalar_mul(
                                ctx[:, qc * 4 + qs, hg * HD:(hg + 1) * HD],
                                cps[h][:, qs * HDA: qs * HDA + HD],
                                rec[:, qs:qs + 1])

            if debug:
                for pt in range(PT):
                    nc.sync.dma_start(dbg_qT[pt * P:(pt + 1) * P, :], qT[:, pt])
                    nc.sync.dma_start(dbg_kT[pt * P:(pt + 1) * P, :], kT[:, pt])
                for kb in range(KB):
                    nc.sync.dma_start(dbg_vaug[kb * P:(kb + 1) * P, :], vaug[:, kb])
                for i in range(S // P):
                    nc.sync.dma_start(dbg_ctx[i * P:(i + 1) * P, :], ctx[:, i])

            # Context out to DRAM, reload transposed, output projection.
            for i in range(S // P):
                nc.sync.dma_start(ctx_dram[i * P:(i + 1) * P, :], ctx[:, i])
            ctxT = persist.tile([P, PT, S], f16)
            for dt in range(PT):
                nc.sync.dma_start_transpose(
                    ctxT[:, dt], ctx_dram[:, dt * P:(dt + 1) * P])
            for sc_ in range(S // P):
                for jc in range(2):
                    ps = proj_psum.tile([P, 512], f32, tag="proj")
                    for dt in range(PT):
                        nc.tensor.matmul(
                            ps[:], ctxT[:, dt, sc_ * P:(sc_ + 1) * P],
                            wo[:, dt, jc * 512:(jc + 1) * 512],
                            start=(dt == 0), stop=(dt == PT - 1))
                    ot = epool.tile([P, 512], f32, tag="o")
                    nc.vector.tensor_copy(ot[:], ps[:])
                    nc.sync.dma_start(
                        out[sc_ * P:(sc_ + 1) * P, jc * 512:(jc + 1) * 512],
                        ot[:])

    nc.compile()
    return nc


def _prep_inputs(query, key, value, Wq, Wk, Wv, Wo):
    """Per-core input maps; host does the transposes and fp16 casts."""
    f16 = np.float16
    in_maps = []
    wT = {}
    for hg in range(2):
        lo, hi = hg * DHG, (hg + 1) * DHG
        wT[hg] = {
            "wqT": np.ascontiguousarray(Wq[lo:hi, :].T).astype(f16),
            "wkT": np.ascontiguousarray(Wk[lo:hi, :].T).astype(f16),
            "wvT": np.ascontiguousarray(Wv[lo:hi, :].T).astype(f16),
            "woT": np.ascontiguousarray(Wo[:, lo:hi].T).astype(f16),
        }
    for c in range(N_CORES):
        b, hg = c // 2, c % 2
        in_maps.append({
            "xqT": np.ascontiguousarray(query[b].T).astype(f16),
            "xkT": np.ascontiguousarray(key[b].T).astype(f16),
            "xvT": np.ascontiguousarray(value[b].T).astype(f16),
            **wT[hg],
        })
    return in_maps


def _reference_numpy(query, key, value, mask, Wq, Wk, Wv, Wo):
    """Correctness fallback for inputs the fast path doesn't handle."""
    out = np.empty((B, S, D), np.float32)
    for b in range(B):
        q = (query[b] @ Wq.T).reshape(S, H, HD).transpose(1, 0, 2)
        k = (key[b] @ Wk.T).reshape(S, H, HD).transpose(1, 0, 2)
        v = (value[b] @ Wv.T).reshape(S, H, HD).transpose(1, 0, 2)
        scores = np.einsum("hqd,hkd->hqk", q, k) / np.sqrt(np.float32(HD))
        scores = np.where(mask[b][None, :, :] == 0, -np.inf, scores)
        scores = scores - scores.max(axis=-1, keepdims=True)
        e = np.exp(scores)
        attn = e / e.sum(axis=-1, keepdims=True)
        ctx = np.einsum("hqk,hkd->hqd", attn, v)
        out[b] = ctx.transpose(1, 0, 2).reshape(S, D) @ Wo.T
    return out


def run_device(query, key, value, Wq, Wk, Wv, Wo, trace=False, trace_kwargs=None,
               debug=False):
    from concourse.bass_utils import run_bass_kernel_spmd

    key_ = ("nc", debug)
    if key_ not in _CACHE:
        _CACHE[key_] = _build_nc(debug)
    nc = _CACHE[key_]
    in_maps = _prep_inputs(query, key, value, Wq, Wk, Wv, Wo)
    res = run_bass_kernel_spmd(nc, in_maps, list(range(N_CORES)),
                               trace=trace, **(trace_kwargs or {}))
    out = np.empty((B, S, D), np.float32)
    for b in range(B):
        out[b] = res.results[2 * b]["out"] + res.results[2 * b + 1]["out"]
    return out, res


def kernel(query, key, value, mask, Wq, Wk, Wv, Wo):
    query = np.asarray(query, np.float32)
    key = np.asarray(key, np.float32)
    value = np.asarray(value, np.float32)
    Wq = np.asarray(Wq, np.float32)
    Wk = np.asarray(Wk, np.float32)
    Wv = np.asarray(Wv, np.float32)
    Wo = np.asarray(Wo, np.float32)
    if not np.all(np.asarray(mask) == 1):
        return _reference_numpy(query, key, value, np.asarray(mask),
                                Wq, Wk, Wv, Wo)
    out, _ = run_device(query, key, value, Wq, Wk, Wv, Wo)
    return out


# revision 18
# speedup vs baseline: 126.5118x; 126.5118x over previous
"""Multi-head attention (B=4, S=2048, D=1024, H=16) on 8 Trainium2 cores.

Sharding: core c computes batch b = c // 2, head group hg = c % 2 (8 heads).
Each core runs the full pipeline for its (b, hg): QKV projections restricted
to its head group's rows of Wq/Wk/Wv, per-head attention, and a partial
output projection against its head group's columns of Wo. The host sums the
two partial outputs per batch (the out-projection is linear in the head dim).

On-device layouts are transposed (feature on partitions):
  qT/kT [512, 2048]: head-dim on partitions; pair tile p holds head 2p on
  partitions 0:64 and head 2p+1 on 64:128, so the two scores matmuls of a
  pair land on disjoint PE row groups and run concurrently.
  Scores are computed as S^T [k, q]; exp(S^T/8) -> E^T feeds the context
  matmul as the *moving* operand with [V | ones] stationary, yielding
  ctx^T [d, q] plus the softmax denominator as row 64 of the same PSUM
  accumulation. Normalization = reciprocal of that row, partition-broadcast
  (GpSimd), one multiply. ctx^T is exactly the lhsT the out-projection
  needs, so no transposes anywhere.
"""

import numpy as np

B, S, D, H = 4, 2048, 1024, 16
HD = D // H          # 64
NHL = 8              # heads per core
DHG = NHL * HD       # 512 head-group width
HDA = HD + 1         # augmented head dim (ones column)
P = 128
N_CORES = 8

_CACHE = {}


def _build_nc(debug=False):
    import concourse.bacc as bacc
    import concourse.mybir as mybir
    from concourse.tile import TileContext

    f16 = mybir.dt.float16
    f32 = mybir.dt.float32
    EXP = mybir.ActivationFunctionType.Exp

    nc = bacc.Bacc("TRN2", target_bir_lowering=False, debug=False,
                   num_devices=N_CORES)

    xqT = nc.dram_tensor("xqT", [D, S], f16, kind="ExternalInput")
    xkT = nc.dram_tensor("xkT", [D, S], f16, kind="ExternalInput")
    xvT = nc.dram_tensor("xvT", [D, S], f16, kind="ExternalInput")
    wqT = nc.dram_tensor("wqT", [D, DHG], f16, kind="ExternalInput")
    wkT = nc.dram_tensor("wkT", [D, DHG], f16, kind="ExternalInput")
    wvT = nc.dram_tensor("wvT", [D, DHG], f16, kind="ExternalInput")
    woT = nc.dram_tensor("woT", [DHG, D], f16, kind="ExternalInput")
    out = nc.dram_tensor("out", [S, D], f32, kind="ExternalOutput")
    if debug:
        dbg_qT = nc.dram_tensor("dbg_qT", [DHG, S], f16, kind="ExternalOutput")
        dbg_kT = nc.dram_tensor("dbg_kT", [DHG, S], f16, kind="ExternalOutput")
        dbg_vaug = nc.dram_tensor("dbg_vaug", [S, NHL * HDA], f16, kind="ExternalOutput")
        dbg_ctxT = nc.dram_tensor("dbg_ctxT", [DHG, S], f16, kind="ExternalOutput")

    DT = D // P          # 8 input-dim tiles
    PT = DHG // P        # 4 head-pair tiles
    QC = S // 512        # 4 query chunks
    KB = S // P          # 16 key chunks

    with TileContext(nc) as tc:
        with (
            tc.tile_pool(name="weights", bufs=1) as wpool,
            tc.tile_pool(name="persist", bufs=1) as persist,
            tc.tile_pool(name="xstream", bufs=16) as xpool,
            tc.tile_pool(name="evict", bufs=6) as epool,
            tc.tile_pool(name="norm", bufs=3) as npool,
            tc.tile_pool(name="proj_psum", bufs=2, space="PSUM") as proj_psum,
            tc.tile_pool(name="sc_psum", bufs=2, space="PSUM") as sc_psum,
            tc.tile_pool(name="ctx_psum", bufs=2, space="PSUM") as ctx_psum,
        ):
            wq = wpool.tile([P, DT, DHG], f16)
            wk = wpool.tile([P, DT, DHG], f16)
            wv = wpool.tile([P, DT, DHG], f16)
            wo = wpool.tile([P, PT, D], f16)
            for dt in range(DT):
                nc.sync.dma_start(wq[:, dt], wqT[dt * P:(dt + 1) * P, :])
                nc.sync.dma_start(wk[:, dt], wkT[dt * P:(dt + 1) * P, :])
                nc.sync.dma_start(wv[:, dt], wvT[dt * P:(dt + 1) * P, :])
            for dt in range(PT):
                nc.sync.dma_start(wo[:, dt], woT[dt * P:(dt + 1) * P, :])

            qT = persist.tile([P, PT, S], f16)
            kT = persist.tile([P, PT, S], f16)
            vaug = persist.tile([P, KB, NHL * HDA], f16)
            ctxT = persist.tile([P, PT, S], f16)

            # K and Q projections.  Each x tile is loaded once (qc-outer)
            # and consumed by all four head-pair output tiles.  K runs
            # first, and Q's pair-0 chunks are emitted first within each
            # qc so attention on pair 0 can start as early as possible.
            def project_kq(w, xdram, dst, only_qc=None):
                for qc in ([only_qc] if only_qc is not None else range(QC)):
                    xs = []
                    for dt in range(DT):
                        xt = xpool.tile([P, 512], f16, tag="x")
                        nc.sync.dma_start(
                            xt[:], xdram[dt * P:(dt + 1) * P,
                                         qc * 512:(qc + 1) * 512])
                        xs.append(xt)
                    for pt in range(PT):
                        ps = proj_psum.tile([P, 512], f32, tag="proj")
                        for dt in range(DT):
                            nc.tensor.matmul(
                                ps[:], w[:, dt, pt * P:(pt + 1) * P], xs[dt][:],
                                start=(dt == 0), stop=(dt == DT - 1))
                        nc.vector.tensor_copy(
                            dst[:, pt, qc * 512:(qc + 1) * 512], ps[:])

            def project_v(kc):
                xs = []
                for dt in range(DT):
                    xt = xpool.tile([P, 512], f16, tag="x")
                    nc.sync.dma_start(
                        xt[:], xvT[dt * P:(dt + 1) * P, kc * 512:(kc + 1) * 512])
                    xs.append(xt)
                for ks in range(4):
                    kb = kc * 4 + ks
                    ps = proj_psum.tile([P, 512], f32, tag="proj")
                    for dt in range(DT):
                        nc.tensor.matmul(
                            ps[:], xs[dt][:, ks * P:(ks + 1) * P], wv[:, dt],
                            start=(dt == 0), stop=(dt == DT - 1))
                    va = vaug[:, kb].rearrange("p (h x) -> p h x", h=NHL)
                    nc.any.memset(va[:, :, HD:HDA], 1.0)
                    nc.vector.tensor_copy(
                        va[:, :, 0:HD],
                        ps[:].rearrange("p (h x) -> p h x", h=NHL))

            # Emission order interleaves the remaining projections with the
            # attention chunks: attention on query chunk qc only needs all
            # of K, Q chunk qc, and V chunks as its key loop reaches them,
            # so ACT starts exponentiating ~50us earlier and the V/Q
            # projection matmuls fill PE slack while ACT is the bottleneck.
            project_kq(wk, xkT, kT)
            for kc in range(QC):
                project_v(kc)
            project_kq(wq, xqT, qT)

            # Attention: qc-outer so each query chunk's output-projection
            # rows can run as soon as all pairs finish that chunk.
            # ctx^T accumulates with [V | ones] stationary: PSUM rows 0:64
            # are ctx^T, row 64 is the softmax denominator per query.
            for qc in range(QC):
                for pr in range(PT):
                    cps = [ctx_psum.tile([P, 512], f32, tag="ctx",
                                         name=f"ctx_{pr}_{qc}_{h}")
                           for h in range(2)]
                    for kb in range(KB):
                        sc = sc_psum.tile([P, 1024], f32, tag="sc")
                        et = epool.tile([P, 1024], f16, tag="e")
                        for h in range(2):
                            nc.tensor.matmul(
                                sc[:, h * 512:(h + 1) * 512],
                                kT[h * 64:(h + 1) * 64, pr, kb * P:(kb + 1) * P],
                                qT[h * 64:(h + 1) * 64, pr, qc * 512:(qc + 1) * 512],
                                start=True, stop=True)
                        nc.scalar.activation(et[:], sc[:], EXP, scale=1.0 / 8.0)
                        for h in range(2):
                            hg = 2 * pr + h
                            nc.tensor.matmul(
                                cps[h][0:HDA, :],
                                vaug[:, kb, hg * HDA:(hg + 1) * HDA],
                                et[:, h * 512:(h + 1) * 512],
                                start=(kb == 0), stop=(kb == KB - 1))
                    for h in range(2):
                        # denominator row lives at PSUM partition 64; the
                        # custom-DVE reciprocal and partition_broadcast only
                        # operate from partition 0, so bounce it via DMA.
                        den = npool.tile([P, 512], f32, tag="den",
                                         name=f"den_{pr}_{qc}_{h}")
                        nc.vector.tensor_copy(den[HD:HDA, :], cps[h][HD:HDA, :])
                        nc.sync.dma_start(den[0:1, :], den[HD:HDA, :])
                        rec = npool.tile([1, 512], f32, tag="rec")
                        nc.vector.reciprocal_approx_fast(rec[0:1, :], den[0:1, :])
                        rb = npool.tile([HD, 512], f32, tag="rb")
                        nc.gpsimd.partition_broadcast(rb[:], rec[0:1, :])
                        if h == 0:
                            nc.vector.tensor_mul(
                                ctxT[0:HD, pr, qc * 512:(qc + 1) * 512],
                                cps[h][0:HD, :], rb[:])
                        else:
                            tmp = npool.tile([HD, 512], f16, tag="tmp")
                            nc.vector.tensor_mul(tmp[:], cps[h][0:HD, :], rb[:])
                            nc.sync.dma_start(
                                ctxT[HD:P, pr, qc * 512:(qc + 1) * 512], tmp[:])

                # Output projection for this query chunk: ctx^T is already
                # the lhsT layout.
                for sq in range(4):
                    sc_ = qc * 4 + sq
                    for jc in range(2):
                        ps = proj_psum.tile([P, 512], f32, tag="proj")
                        for dt in range(PT):
                            nc.tensor.matmul(
                                ps[:], ctxT[:, dt, sc_ * P:(sc_ + 1) * P],
                                wo[:, dt, jc * 512:(jc + 1) * 512],
                                start=(dt == 0), stop=(dt == PT - 1))
                        ot = epool.tile([P, 512], f32, tag="o")
                        nc.vector.tensor_copy(ot[:], ps[:])
                        nc.sync.dma_start(
                            out[sc_ * P:(sc_ + 1) * P, jc * 512:(jc + 1) * 512],
                            ot[:])

            if debug:
                for pt in range(PT):
                    nc.sync.dma_start(dbg_qT[pt * P:(pt + 1) * P, :], qT[:, pt])
                    nc.sync.dma_start(dbg_kT[pt * P:(pt + 1) * P, :], kT[:, pt])
                    nc.sync.dma_start(dbg_ctxT[pt * P:(pt + 1) * P, :], ctxT[:, pt])
                for kb in range(KB):
                    nc.sync.dma_start(dbg_vaug[kb * P:(kb + 1) * P, :], vaug[:, kb])

    nc.compile()
    return nc


def _prep_inputs(query, key, value, Wq, Wk, Wv, Wo):
    """Per-core input maps; host does the transposes and fp16 casts."""
    f16 = np.float16
    in_maps = []
    wT = {}
    for hg in range(2):
        lo, hi = hg * DHG, (hg + 1) * DHG
        wT[hg] = {
            "wqT": np.ascontiguousarray(Wq[lo:hi, :].T).astype(f16),
            "wkT": np.ascontiguousarray(Wk[lo:hi, :].T).astype(f16),
            "wvT": np.ascontiguousarray(Wv[lo:hi, :].T).astype(f16),
            "woT": np.ascontiguousarray(Wo[:, lo:hi].T).astype(f16),
        }
    for c in range(N_CORES):
        b, hg = c // 2, c % 2
        in_maps.append({
            "xqT": np.ascontiguousarray(query[b].T).astype(f16),
            "xkT": np.ascontiguousarray(key[b].T).astype(f16),
            "xvT": np.ascontiguousarray(value[b].T).astype(f16),
            **wT[hg],
        })
    return in_maps


def _reference_numpy(query, key, value, mask, Wq, Wk, Wv, Wo):
    """Correctness fallback for inputs the fast path doesn't handle."""
    out = np.empty((B, S, D), np.float32)
    for b in range(B):
        q = (query[b] @ Wq.T).reshape(S, H, HD).transpose(1, 0, 2)
        k = (key[b] @ Wk.T).reshape(S, H, HD).transpose(1, 0, 2)
        v = (value[b] @ Wv.T).reshape(S, H, HD).transpose(1, 0, 2)
        scores = np.einsum("hqd,hkd->hqk", q, k) / np.sqrt(np.float32(HD))
        scores = np.where(mask[b][None, :, :] == 0, -np.inf, scores)
        scores = scores - scores.max(axis=-1, keepdims=True)
        e = np.exp(scores)
        attn = e / e.sum(axis=-1, keepdims=True)
        ctx = np.einsum("hqk,hkd->hqd", attn, v)
        out[b] = ctx.transpose(1, 0, 2).reshape(S, D) @ Wo.T
    return out


def run_device(query, key, value, Wq, Wk, Wv, Wo, trace=False, trace_kwargs=None,
               debug=False):
    from concourse.bass_utils import run_bass_kernel_spmd

    key_ = ("nc", debug)
    if key_ not in _CACHE:
        _CACHE[key_] = _build_nc(debug)
    nc = _CACHE[key_]
    in_maps = _prep_inputs(query, key, value, Wq, Wk, Wv, Wo)
    res = run_bass_kernel_spmd(nc, in_maps, list(range(N_CORES)),
                               trace=trace, **(trace_kwargs or {}))
    out = np.empty((B, S, D), np.float32)
    for b in range(B):
        out[b] = res.results[2 * b]["out"] + res.results[2 * b + 1]["out"]
    return out, res


def kernel(query, key, value, mask, Wq, Wk, Wv, Wo):
    query = np.asarray(query, np.float32)
    key = np.asarray(key, np.float32)
    value = np.asarray(value, np.float32)
    Wq = np.asarray(Wq, np.float32)
    Wk = np.asarray(Wk, np.float32)
    Wv = np.asarray(Wv, np.float32)
    Wo = np.asarray(Wo, np.float32)
    if not np.all(np.asarray(mask) == 1):
        return _reference_numpy(query, key, value, np.asarray(mask),
                                Wq, Wk, Wv, Wo)
    out, _ = run_device(query, key, value, Wq, Wk, Wv, Wo)
    return out
